# revision 11
# baseline (speedup 1.0000x reference)
"""Bass/Trainium2 kernel for nn_LocalAggregator (GNN message passing), v8.

Math per batch b (hidden [64,128], adj [64,64] in {0..4}, a [4,128]):
    e_k[i,j] = leakyrelu_{0.2}( sum_d hidden[i,d]*hidden[j,d]*a[k,d] )
    alpha    = softmax_j( where(adj==k+1, e_k, -9e15) )
    out      = alpha @ hidden

v8: 8 batches per iteration ("oct"), select-FIRST in the log domain.
  - e_k symmetric in (i,j): mask with host-TRANSPOSED adjacency to get
    transposed attention weights directly (no on-chip transposes).
  - One-hot select runs on the leakyrelu'd logits (DVE, bf16), k-summed
    down to ONE [64,64] plane per batch BEFORE exp, so the ACT engine
    exps 256 cols/oct instead of 1024 (ACT was the saturated engine).
  - Masked (adj==0) entries select to 0 -> exp gives 1; a host-shipped
    -1/0 matrix rides a second accumulating matmul into the output so
    both the numerator and the ones-column denominator are corrected.
  - Outputs ship RAW (numerator + denominator col) in bf16; the
    gather/unshard step divides on host.  GPSIMD is never used.
"""

import numpy as np
import ml_dtypes

from contextlib import ExitStack

import concourse.bass as bass
import concourse.tile as tile
from concourse import bacc, mybir
from concourse._compat import with_exitstack
from concourse.bass_utils import run_bass_kernel_spmd

BF16 = mybir.dt.bfloat16
F32 = mybir.dt.float32
ALU = mybir.AluOpType
ACTF = mybir.ActivationFunctionType

B, N, D, K = 512, 64, 128, 4
NCORES = 8
BPC = B // NCORES          # 64 batches per core
OCTS = BPC // 8            # 8 octs of 8 batches per core
HHW = 132                  # hidden cols + ones col + pad
# fused input cols: hT 0:512 | hh 512:1040 | ind 1040:2064 | m0 2064:2320
HT0, HH0, IND0, M00 = 0, 512, 1040, 2064
INW = 2320
OPW = 2 * HHW              # 264: one PSUM output tile covers 2 batch-pairs


@with_exitstack
def _kernel_body(ctx, tc, in_d, abc_d, out_d):
    nc = tc.nc

    const_pool = ctx.enter_context(tc.tile_pool(name="const", bufs=1))
    in_pool = ctx.enter_context(tc.tile_pool(name="inp", bufs=8))
    work_pool = ctx.enter_context(tc.tile_pool(name="work", bufs=3))
    psum_pool = ctx.enter_context(tc.tile_pool(name="psum", bufs=2, space="PSUM"))
    opsum_pool = ctx.enter_context(tc.tile_pool(name="opsum", bufs=2, space="PSUM"))
    out_pool = ctx.enter_context(tc.tile_pool(name="outp", bufs=3))

    # a_bc[d, (k,j)] = a[k,d] replicated over j (per-k contiguous blocks)
    a_bc = const_pool.tile([128, 4 * 64], BF16)
    nc.sync.dma_start(out=a_bc[:], in_=abc_d[:, :])

    stage = {}

    def dma_in(o):
        """Prefetch oct o's fused input [SP]."""
        it = in_pool.tile([128, INW], BF16, tag="in", bufs=8)
        nc.sync.dma_start(out=it[:], in_=in_d[o])
        stage[o] = {"it": it}

    def stage_a(o):
        """w_all build [DVE]: w_all[d,(l,k,j)] = hT[d,(l,j)] * a[k,d]."""
        it = stage[o]["it"]
        hT = it[:, HT0:HT0 + 512]                   # [d, (l,i)]
        w_all = work_pool.tile([128, 2048], BF16, tag="w_all", bufs=4)
        w_allv = w_all[:].rearrange("p (l k j) -> p l k j", l=8, k=4)
        hTv = hT.rearrange("p (l j) -> p l j", l=8)
        abv = a_bc[:].rearrange("p (k j) -> p k j", k=4)
        nc.vector.tensor_tensor(
            w_allv,
            hTv.unsqueeze(2).broadcast_to([128, 8, 4, 64]),
            abv.unsqueeze(1).broadcast_to([128, 8, 4, 64]),
            ALU.mult)
        stage[o]["w_all"] = w_all

    def stage_b(o):
        """e-matmuls [PE]: e4[(u,i), (p,k,j)], p in 0..3, contraction 128."""
        st = stage[o]
        hT = st["it"][:, HT0:HT0 + 512]
        w_all = st.pop("w_all")
        e4 = psum_pool.tile([128, 1024], F32, tag="e4", bufs=2)
        e4v = e4[:].rearrange("a (p k j) -> a p k j", p=4, k=4)
        for l in range(8):
            p, u = l // 2, l % 2
            nc.tensor.matmul(
                e4v[u * 64:(u + 1) * 64, p],
                lhsT=hT[:, l * 64:(l + 1) * 64],
                rhs=w_all[:, l * 256:(l + 1) * 256],
                start=True, stop=True,
                tile_position=(0, u * 64),
            )
        st["e4"] = e4

    def stage_c(o):
        """lr4 = leakyrelu(e4) [ACT], bf16."""
        st = stage[o]
        lr4 = work_pool.tile([128, 1024], BF16, tag="lr4", bufs=3)
        nc.scalar.activation(lr4[:], st.pop("e4")[:], ACTF.Prelu, alpha=0.2)
        st["lr4"] = lr4

    def stage_d(o):
        """one-hot select + k-sum in log domain [DVE]."""
        st = stage[o]
        it, lr4 = st["it"], st.pop("lr4")
        sel = work_pool.tile([128, 1024], BF16, tag="sel", bufs=3)
        nc.vector.tensor_mul(sel[:], lr4[:], it[:, IND0:IND0 + 1024])
        selv = sel[:].rearrange("p (t k c) -> p t k c", t=4, k=4)
        t2 = work_pool.tile([128, 512], BF16, tag="t2", bufs=3)
        t2v = t2[:].rearrange("p (t k c) -> p t k c", t=4, k=2)
        nc.vector.tensor_tensor(t2v, selv[:, :, 0:2, :], selv[:, :, 2:4, :], ALU.add)
        lrs = work_pool.tile([128, 256], BF16, tag="lrs", bufs=4)
        lrsv = lrs[:].rearrange("p (t c) -> p t c", t=4)
        nc.vector.tensor_tensor(lrsv, t2v[:, :, 0, :], t2v[:, :, 1, :], ALU.add)
        st["lrs"] = lrs

    def stage_e(o):
        """xm = exp(selected logits) [ACT], then +M0 fix [DVE]: masked
        entries exp to exactly 1.0 and the -1/0 m0 add zeroes them."""
        st = stage[o]
        xm = work_pool.tile([128, 256], BF16, tag="xm", bufs=4)
        nc.scalar.activation(xm[:], st.pop("lrs")[:], ACTF.Exp)
        wadj = work_pool.tile([128, 256], BF16, tag="wadj", bufs=4)
        nc.vector.tensor_tensor(wadj[:], xm[:], st["it"][:, M00:M00 + 256],
                                ALU.add)
        st["wadj"] = wadj

    def stage_f(o):
        """out matmuls [PE]: (xm - M0) @ hh."""
        st = stage[o]
        it, wadj = st["it"], st.pop("wadj")
        for ph in range(2):
            ops = opsum_pool.tile([128, OPW], F32, tag=f"ops{ph}", bufs=2)
            for pl in range(2):
                p = 2 * ph + pl
                for u in range(2):
                    nc.tensor.matmul(
                        ops[u * 64:(u + 1) * 64, pl * HHW:(pl + 1) * HHW],
                        lhsT=wadj[u * 64:(u + 1) * 64, p * 64:(p + 1) * 64],
                        rhs=it[u * 64:(u + 1) * 64,
                               HH0 + p * HHW:HH0 + (p + 1) * HHW],
                        start=True, stop=True,
                        tile_position=(u * 64, u * 64),
                    )
            st[f"ops{ph}"] = ops

    def stage_g(o):
        """evacuate PSUM [ACT, bf16] + DMA out raw+denominator [SP]."""
        st = stage.pop(o)
        osb = out_pool.tile([128, 2 * OPW], BF16, tag="osb", bufs=3)
        for ph in range(2):
            nc.scalar.activation(osb[:, ph * OPW:(ph + 1) * OPW],
                                 st[f"ops{ph}"][:], ACTF.Copy)
        nc.sync.dma_start(out=out_d[o], in_=osb[:])

    stages = [dma_in, stage_a, stage_b, stage_c, stage_d, stage_e,
              stage_f, stage_g]
    for i in range(OCTS + len(stages) - 1):
        for s_idx, fn in enumerate(stages):
            o = i - s_idx
            if 0 <= o < OCTS:
                fn(o)


def build_nc():
    nc = bacc.Bacc("TRN2", target_bir_lowering=False, debug=False)
    in_d = nc.dram_tensor("inp", [OCTS, 128, INW], BF16, kind="ExternalInput").ap()
    abc_d = nc.dram_tensor("abc", [128, 256], BF16, kind="ExternalInput").ap()
    out_d = nc.dram_tensor("out", [OCTS, 128, 2 * OPW], BF16,
                           kind="ExternalOutput").ap()
    with tile.TileContext(nc) as tc:
        _kernel_body(tc, in_d, abc_d, out_d)
    nc.compile()
    return nc


def prep_inputs(hidden, adj, a):
    """Host-side packing: bf16 casts, fused oct layout, one-hot mask."""
    bf = ml_dtypes.bfloat16
    hidden = np.asarray(hidden, dtype=np.float32)
    adj = np.asarray(adj)
    a = np.asarray(a, dtype=np.float32)

    hb = hidden.astype(bf)                                   # [B, 64, 128]
    no = B // 8

    fused = np.zeros((no, 128, INW), dtype=bf)

    # hT[o, d, l*64+i] = hidden[8o+l, i, d]
    fused[:, :, HT0:HT0 + 512] = (hb.transpose(0, 2, 1)      # [B, d, i]
                                  .reshape(no, 8, D, N)      # [o, l, d, i]
                                  .transpose(0, 2, 1, 3)     # [o, d, l, i]
                                  .reshape(no, D, 8 * N))

    # hh[o, u*64+j, p*HHW+c] = hidden[8o+2p+u, j, c]; ones col at c=128
    hq = (hb.reshape(no, 4, 2, N, D)                         # [o, p, u, j, c]
          .transpose(0, 2, 3, 1, 4)                          # [o, u, j, p, c]
          .reshape(no, 128, 4, D))
    for p in range(4):
        fused[:, :, HH0 + p * HHW:HH0 + p * HHW + D] = hq[:, :, p, :]
        fused[:, :, HH0 + p * HHW + D] = bf(1.0)

    # adjq[o, u, r, p, c] = adj[8o+2p+u][c, r]  (transposed adjacency)
    adjT = adj.transpose(0, 2, 1)                            # [b, r, c]
    adjq = (adjT.reshape(no, 4, 2, N, N)                     # [o, p, u, r, c]
            .transpose(0, 2, 3, 1, 4))                       # [o, u, r, p, c]

    # ind[o, u*64+r, p*256+k*64+c] = (adjq == k+1)
    ind = np.zeros((no, 2, N, 4, K, N), dtype=bf)            # [o, u, r, p, k, c]
    for k in range(K):
        ind[:, :, :, :, k, :] = (adjq == k + 1)
    fused[:, :, IND0:IND0 + 1024] = ind.reshape(no, 128, 1024)

    # m0[o, u*64+r, p*64+c] = -1 where adjq==0 (exp(0)=1 correction)
    m0 = -(adjq == 0).astype(bf)                             # [o, u, r, p, c]
    fused[:, :, M00:M00 + 256] = m0.reshape(no, 128, 256)

    # a_bc[d, k*64+j] = a[k, d]
    abc = np.ascontiguousarray(
        np.repeat(a.T.astype(bf)[:, :, None], 64, axis=2).reshape(128, 256))

    in_maps = []
    for c in range(NCORES):
        osl = slice(c * OCTS, (c + 1) * OCTS)
        in_maps.append({
            "inp": np.ascontiguousarray(fused[osl]),
            "abc": abc,
        })
    return in_maps


_NC_CACHE = {}


def run_device(hidden, adj, a, **spmd_kwargs):
    if "nc" not in _NC_CACHE:
        _NC_CACHE["nc"] = build_nc()
    nc = _NC_CACHE["nc"]
    in_maps = prep_inputs(hidden, adj, a)
    res = run_bass_kernel_spmd(nc, in_maps, list(range(NCORES)), **spmd_kwargs)
    # out[o, u*64+i, ph*OPW + pl*HHW + c] -> batch 8o+2(2ph+pl)+u, row i,
    # col c; col 128 is the softmax denominator (divide on host).
    outs = []
    for c in range(NCORES):
        o = res.results[c]["out"].astype(np.float32)         # [O, 128, 528]
        o = o.reshape(OCTS, 2, N, 2, 2, HHW)                 # [o, u, i, ph, pl, c]
        o = o.transpose(0, 3, 4, 1, 2, 5)                    # [o, ph, pl, u, i, c]
        o = o.reshape(BPC, N, HHW)
        outs.append(o[:, :, 0:D] / o[:, :, D:D + 1])
    out = np.concatenate(outs, axis=0)
    return out.astype(np.float32), res


def kernel(hidden, adj, a):
    out, _ = run_device(hidden, adj, a)
    return out
